# revision 1
# baseline (speedup 1.0000x reference)
"""GATv2 3-layer kernel for 8 TRN2 NeuronCores (Bass/Tile).

Dst-sharded: each core owns 12500 dst nodes, replicates the tiny dense
transforms for all nodes into a local DRAM gather table, then runs a
dst-major edge phase (dma_gather of per-edge source rows, DVE softmax +
weighted sum), PE-transposes layer outputs to feature-major shards and
AllGathers them between layers.

All per-core variation is in input data (index lists, local x columns);
the traced program is identical across cores (SPMD). att is folded into
the weights (u = |att|*(xl+xr)) with a sign-split min/max leaky-relu;
padded slots point at per-chunk magic rows (+-1000) so exp -> 0.
"""
import sys

sys.path.insert(0, "/opt/trn_rl_repo")

import numpy as np

N = 100000
NCORES = 8
SH = 12500
PSH = 12544                 # 98 * 128
NBLK = 98
NN = NCORES * PSH           # 100352
CSTRIDE = 25089             # chunk stride in table rows (incl magic row)
CNODES = 25088              # real rows per chunk (2 core shards)
NCHUNK = 4
NTAB = NCHUNK * CSTRIDE
MAGIC_LOCAL = CNODES
DIMS = [(11, 16), (16, 32), (32, 64)]
GRP = 896                   # dense-phase node group (7*128)
NGRP = PSH // GRP           # 14

TRACE = False
LAST_EXEC_NS = None


def _preprocess(edge_index):
    src = np.concatenate([edge_index[0].astype(np.int64), np.arange(N, dtype=np.int64)])
    dst = np.concatenate([edge_index[1].astype(np.int64), np.arange(N, dtype=np.int64)])
    node_owner = np.arange(N) // SH
    node_chunk = node_owner // 2

    cnt = np.zeros((N, NCHUNK), dtype=np.int32)
    np.add.at(cnt, (dst, node_chunk[src]), 1)

    localpos = np.empty(N, dtype=np.int64)
    order_per_core = []
    for c in range(NCORES):
        nodes = np.arange(c * SH, (c + 1) * SH)
        cc = cnt[nodes]
        o = np.lexsort((cc[:, 3], cc[:, 2], cc[:, 1], cc[:, 0]))[::-1]
        nodes = nodes[o]
        order_per_core.append(nodes)
        localpos[nodes] = np.arange(SH)

    tabrow = (node_owner // 2) * CSTRIDE + (node_owner % 2) * PSH + localpos

    cntp = np.zeros((NCORES, NBLK, 128, NCHUNK), dtype=np.int32)
    for c in range(NCORES):
        cc = cnt[order_per_core[c]]
        cc = np.concatenate([cc, np.zeros((PSH - SH, NCHUNK), np.int32)], 0)
        cntp[c] = cc.reshape(NBLK, 128, NCHUNK)
    D = cntp.max(axis=(0, 2)).astype(np.int64)      # [NBLK, NCHUNK]
    Dtot = D.sum(axis=1)                            # [NBLK]
    SDT = int(Dtot.sum())

    blk_base = np.r_[0, np.cumsum(Dtot)][:-1]
    coloff = np.zeros((NBLK, NCHUNK), dtype=np.int64)
    for b in range(NBLK):
        coloff[b] = blk_base[b] + np.r_[0, np.cumsum(D[b])][:-1]

    # slot grid [core, 128, SDT], value = chunk-local table row of src
    ecore = dst // SH
    edl = localpos[dst]
    eblk, epart = edl // 128, edl % 128
    echunk = node_chunk[src]
    eval_loc = tabrow[src] - echunk * CSTRIDE
    key = ((ecore * NBLK + eblk) * 128 + epart) * NCHUNK + echunk
    eo = np.argsort(key, kind='stable')
    keys, vals = key[eo], eval_loc[eo]
    grp_start = np.r_[0, np.flatnonzero(np.diff(keys)) + 1]
    grp_len = np.diff(np.r_[grp_start, len(keys)])
    jidx = np.arange(len(keys)) - np.repeat(grp_start, grp_len)
    kc = keys % NCHUNK
    kp = (keys // NCHUNK) % 128
    kb = (keys // (NCHUNK * 128)) % NBLK
    kcore = keys // (NCHUNK * 128 * NBLK)
    slots = np.full((NCORES, 128, SDT), MAGIC_LOCAL, dtype=np.int64)
    slots[kcore, kp, coloff[kb, kc] + jidx] = vals

    # wrapped int16 per (block, chunk) call, concatenated along free dim
    idx_flat = np.empty((NCORES, 128, 8 * SDT), dtype=np.int16)
    for b in range(NBLK):
        for ch in range(NCHUNK):
            w = int(D[b, ch])
            if w == 0:
                continue
            cs = int(coloff[b, ch])
            sub = slots[:, :, cs:cs + w]                          # [NC,128,w]
            lst = sub.transpose(0, 2, 1).reshape(NCORES, w * 128)  # pos=j*128+p
            wr = lst.reshape(NCORES, w * 8, 16).transpose(0, 2, 1)
            wr = np.tile(wr, (1, 8, 1))
            idx_flat[:, :, 8 * cs:8 * (cs + w)] = wr.astype(np.int16)

    meta = dict(D=D, Dtot=Dtot, coloff=coloff, blk_base=blk_base,
                order_per_core=order_per_core, SDT=SDT)
    return idx_flat, meta


def _build_program(meta, kpos_list):
    import concourse.bass as bass
    import concourse.bacc as bacc
    import concourse.tile as tile
    import concourse.mybir as mybir
    from concourse import masks

    D, Dtot, coloff = meta['D'], meta['Dtot'], meta['coloff']
    blk_base, SDT = meta['blk_base'], meta['SDT']
    f32 = mybir.dt.float32
    i16 = mybir.dt.int16
    AF = mybir.ActivationFunctionType
    OP = mybir.AluOpType
    AX = mybir.AxisListType

    nc = bacc.Bacc("TRN2", target_bir_lowering=False, debug=False,
                   num_devices=NCORES)
    t_xT = nc.dram_tensor("xT", [11, NN], f32, kind="ExternalInput")
    t_xTloc = nc.dram_tensor("xTloc", [11, PSH], f32, kind="ExternalInput")
    t_idx = nc.dram_tensor("idxf", [1, 128 * 8 * SDT], i16, kind="ExternalInput")
    t_Wl, t_Wr, t_bxr, t_invs, t_obias, t_magic = [], [], [], [], [], []
    for l in range(3):
        din, dout = DIMS[l]
        t_Wl.append(nc.dram_tensor(f"Wl{l}", [din, 64], f32, kind="ExternalInput"))
        t_Wr.append(nc.dram_tensor(f"Wr{l}", [din, 64], f32, kind="ExternalInput"))
        t_bxr.append(nc.dram_tensor(f"bxr{l}", [1, 64], f32, kind="ExternalInput"))
        t_invs.append(nc.dram_tensor(f"invs{l}", [1, 64], f32, kind="ExternalInput"))
        shape = [1, 64] if l == 2 else [dout, 1]
        t_obias.append(nc.dram_tensor(f"obias{l}", shape, f32, kind="ExternalInput"))
        t_magic.append(nc.dram_tensor(f"magic{l}", [1, 64], f32, kind="ExternalInput"))
    t_out = nc.dram_tensor("out", [PSH, 64], f32, kind="ExternalOutput")

    with tile.TileContext(nc) as tc:
        with (tc.tile_pool(name="const", bufs=1) as cpool,
              tc.tile_pool(name="resident", bufs=1) as rpool,
              tc.tile_pool(name="dram", bufs=1, space="DRAM") as dpool,
              tc.tile_pool(name="work", bufs=2) as wpool,
              tc.tile_pool(name="feed", bufs=3) as fpool,
              tc.tile_pool(name="small", bufs=4) as spool,
              tc.tile_pool(name="psum", bufs=2, space="PSUM") as ppool,
              tc.tile_pool(name="psumT", bufs=2, space="PSUM") as ppoolT):

            t_tab = [dpool.tile([NTAB, 64], f32, name=f"tab{l}") for l in range(3)]
            t_agin = [dpool.tile([DIMS[l][1], PSH], f32,
                                 name=f"agin{l}") for l in range(2)]
            t_agout = [dpool.tile([NCORES, DIMS[l][1], PSH], f32,
                                  addr_space="Shared", name=f"agout{l}")
                       for l in range(2)]

            ident = cpool.tile([128, 128], f32)
            masks.make_identity(nc, ident[:, :])
            ones_row = cpool.tile([1, 128], f32)
            nc.vector.memset(ones_row[:, :], 1.0)

            def replicate_row(src_row, name):
                ps = ppoolT.tile([128, 64], f32, tag="repl")
                nc.tensor.matmul(ps[:, :], ones_row[:, :], src_row[:, :])
                rep = cpool.tile([128, 64], f32, name=name)
                nc.scalar.activation(rep[:, :], ps[:, :], AF.Copy)
                return rep

            c_bxr, c_invs, c_obias, c_W = [], [], [], []
            for l in range(3):
                din = DIMS[l][0]
                r = cpool.tile([1, 64], f32, name=f"r1_{l}")
                nc.sync.dma_start(r[:, :], t_bxr[l][:, :])
                c_bxr.append(replicate_row(r, f"bxr_{l}"))
                r2 = cpool.tile([1, 64], f32, name=f"r2_{l}")
                nc.sync.dma_start(r2[:, :], t_invs[l][:, :])
                c_invs.append(replicate_row(r2, f"invs_{l}"))
                if l == 2:
                    r3 = cpool.tile([1, 64], f32, name=f"r3_{l}")
                    nc.sync.dma_start(r3[:, :], t_obias[l][:, :])
                    c_obias.append(replicate_row(r3, f"obias_{l}"))
                else:
                    col = cpool.tile([DIMS[l][1], 1], f32, name=f"obias_{l}")
                    nc.sync.dma_start(col[:, :], t_obias[l][:, :])
                    c_obias.append(col)
                mg = cpool.tile([1, 64], f32, name=f"mg_{l}")
                nc.sync.dma_start(mg[:, :], t_magic[l][:, :])
                for ch in range(NCHUNK):
                    row = ch * CSTRIDE + CNODES
                    nc.sync.dma_start(t_tab[l][row:row + 1, :], mg[:, :])
                wl = cpool.tile([din, 64], f32, name=f"cWl{l}")
                nc.sync.dma_start(wl[:, :], t_Wl[l][:, :])
                wr = cpool.tile([din, 64], f32, name=f"cWr{l}")
                nc.sync.dma_start(wr[:, :], t_Wr[l][:, :])
                c_W.append((wl, wr))

            xr_res = rpool.tile([128, NBLK * 64], f32)
            hT = [rpool.tile([DIMS[i][1], PSH], f32, name=f"hT{i}") for i in range(2)]

            for l in range(3):
                din, dout = DIMS[l]
                kpos = kpos_list[l]
                wl_t, wr_t = c_W[l]

                # ---- dense: xl'' table for all nodes ----
                for G in range(NCORES * NGRP):
                    shard, g = divmod(G, NGRP)
                    hsl = fpool.tile([din, GRP], f32, tag="hsl")
                    if l == 0:
                        nc.sync.dma_start(
                            hsl[:, :],
                            t_xT[:, shard * PSH + g * GRP:shard * PSH + (g + 1) * GRP])
                    else:
                        nc.sync.dma_start(
                            hsl[:, :],
                            t_agout[l - 1][shard, :, g * GRP:(g + 1) * GRP])
                    ps = ppool.tile([128, 448], f32, tag="psd")
                    for j in range(7):
                        nc.tensor.matmul(ps[:, j * 64:(j + 1) * 64],
                                         hsl[:, j * 128:(j + 1) * 128], wl_t[:, :])
                    sb = wpool.tile([128, 448], f32, tag="sbd")
                    nc.scalar.activation(sb[:, :], ps[:, :], AF.Copy)
                    row0 = (shard // 2) * CSTRIDE + (shard % 2) * PSH + g * GRP
                    nc.sync.dma_start(
                        t_tab[l][row0:row0 + GRP, :].rearrange(
                            "(j p) k -> p j k", p=128),
                        sb.rearrange("p (j k) -> p j k", k=64))

                # ---- dense: xr'' for local shard into xr_res ----
                for g in range(NGRP):
                    hsl = fpool.tile([din, GRP], f32, tag="hsl")
                    if l == 0:
                        nc.sync.dma_start(hsl[:, :],
                                          t_xTloc[:, g * GRP:(g + 1) * GRP])
                    else:
                        nc.sync.dma_start(hsl[:, :],
                                          hT[(l - 1) % 2][0:din, g * GRP:(g + 1) * GRP])
                    ps = ppool.tile([128, 448], f32, tag="psd")
                    for j in range(7):
                        nc.tensor.matmul(ps[:, j * 64:(j + 1) * 64],
                                         hsl[:, j * 128:(j + 1) * 128], wr_t[:, :])
                    nc.vector.tensor_tensor(
                        xr_res.rearrange("p (b k) -> p b k", k=64)[:, 7 * g:7 * g + 7, :],
                        ps.rearrange("p (b k) -> p b k", k=64),
                        c_bxr[l].unsqueeze(1).broadcast_to((128, 7, 64)),
                        OP.add)

                if l > 0:
                    pass
                if l < 2:
                    nc.vector.memset(hT[l % 2][:, :], 0.0)

                # ---- edge phase ----
                for b in range(NBLK):
                    dt = int(Dtot[b])
                    bb = int(blk_base[b])
                    idx_t = fpool.tile([128, 8 * dt], i16, tag="idx")
                    nc.sync.dma_start(
                        idx_t[:, :],
                        t_idx[0, 128 * 8 * bb:128 * 8 * (bb + dt)].rearrange(
                            "(p f) -> p f", p=128))
                    u = wpool.tile([128, dt * 64], f32, tag="u")
                    u3 = u.rearrange("p (d k) -> p d k", d=dt)
                    for ch in range(NCHUNK):
                        w = int(D[b, ch])
                        if w == 0:
                            continue
                        off = int(coloff[b, ch] - blk_base[b])
                        nc.gpsimd.dma_gather(
                            u3[:, off:off + w, :],
                            t_tab[l][ch * CSTRIDE:(ch + 1) * CSTRIDE, :],
                            idx_t[:, 8 * off:8 * (off + w)],
                            num_idxs=128 * w, num_idxs_reg=128 * w,
                            elem_size=64, single_packet=False)
                    xr_blk = xr_res[:, b * 64:(b + 1) * 64]
                    lr = wpool.tile([128, dt * dout], f32, tag="lr")
                    lr3 = lr.rearrange("p (d k) -> p d k", d=dt)
                    e = spool.tile([128, dt], f32, tag="e")
                    eN = spool.tile([128, dt], f32, tag="eN")
                    for ch in range(NCHUNK):
                        w = int(D[b, ch])
                        if w == 0:
                            continue
                        off = int(coloff[b, ch] - blk_base[b])
                        ur = u3[:, off:off + w, :]
                        lrr = lr3[:, off:off + w, :]
                        nc.vector.tensor_tensor(
                            ur, ur,
                            xr_blk.unsqueeze(1).broadcast_to((128, w, 64)),
                            OP.add)
                        if kpos > 0:
                            nc.vector.scalar_tensor_tensor(
                                lrr[:, :, 0:kpos], ur[:, :, 0:kpos], 0.2,
                                ur[:, :, 0:kpos], OP.mult, OP.max)
                            nc.vector.tensor_reduce(
                                e[:, off:off + w], lrr[:, :, 0:kpos],
                                AX.X, OP.add)
                        if kpos < dout:
                            nc.vector.scalar_tensor_tensor(
                                lrr[:, :, kpos:dout], ur[:, :, kpos:dout], 0.2,
                                ur[:, :, kpos:dout], OP.mult, OP.max)
                            nc.vector.tensor_reduce(
                                eN[:, off:off + w], lrr[:, :, kpos:dout],
                                AX.X, OP.add)
                    if 0 < kpos < dout:
                        nc.vector.tensor_tensor(e[:, :], e[:, :], eN[:, :],
                                                OP.subtract)
                    elif kpos == 0:
                        nc.vector.tensor_scalar_mul(e[:, :], eN[:, :], -1.0)
                    m = spool.tile([128, 1], f32, tag="m")
                    nc.vector.tensor_reduce(m[:, :], e[:, :], AX.X, OP.max)
                    negm = spool.tile([128, 1], f32, tag="negm")
                    nc.vector.tensor_scalar_mul(negm[:, :], m[:, :], -1.0)
                    p = spool.tile([128, dt], f32, tag="p")
                    nc.scalar.activation(p[:, :], e[:, :], AF.Exp,
                                         bias=negm[:, :])
                    den = spool.tile([128, 1], f32, tag="den")
                    nc.vector.tensor_reduce(den[:, :], p[:, :], AX.X, OP.add)
                    rden = spool.tile([128, 1], f32, tag="rden")
                    nc.vector.reciprocal(rden[:, :], den[:, :])
                    wg = wpool.tile([128, dt * dout], f32, tag="lr")
                    wg3 = wg.rearrange("p (d k) -> p d k", d=dt)
                    nc.vector.tensor_tensor(
                        wg3, u3[:, :, 0:dout],
                        p.unsqueeze(2).broadcast_to((128, dt, dout)), OP.mult)
                    outU = spool.tile([128, dout], f32, tag="outU")
                    nc.vector.tensor_reduce(outU[:, :],
                                            wg.rearrange("p (d k) -> p k d", d=dt),
                                            AX.X, OP.add)
                    o1 = spool.tile([128, dout], f32, tag="o1")
                    nc.vector.scalar_tensor_tensor(
                        o1[:, :], outU[:, :], rden[:, :], xr_blk[:, 0:dout],
                        OP.mult, OP.subtract)
                    o2 = spool.tile([128, dout], f32, tag="o2")
                    nc.vector.tensor_tensor(o2[:, :], o1[:, :],
                                            c_invs[l][:, 0:dout], OP.mult)
                    if l < 2:
                        trp = ppoolT.tile([64, 128], f32, tag="trp")
                        nc.tensor.transpose(trp[0:dout, :], o2[:, :], ident[:, :])
                        nc.scalar.activation(
                            hT[l % 2][0:dout, b * 128:(b + 1) * 128],
                            trp[0:dout, :], AF.Relu, bias=c_obias[l][:, :])
                    else:
                        o3 = spool.tile([128, 64], f32, tag="o3")
                        nc.vector.tensor_tensor(o3[:, :], o2[:, :],
                                                c_obias[l][:, :], OP.add)
                        nc.sync.dma_start(t_out[b * 128:(b + 1) * 128, :],
                                          o3[:, :])

                if l < 2:
                    dout_l = DIMS[l][1]
                    nc.sync.dma_start(t_agin[l][:, :], hT[l % 2][0:dout_l, :])
                    nc.gpsimd.collective_compute(
                        "AllGather", OP.bypass,
                        replica_groups=[list(range(NCORES))],
                        ins=[t_agin[l].opt()], outs=[t_agout[l].opt()])
    nc.compile()
    return nc


def _prep_inputs(inputs, meta):
    x = np.asarray(inputs["x"], np.float32)
    order = meta['order_per_core']
    xT = np.zeros((11, NN), np.float32)
    for c in range(NCORES):
        xT[:, c * PSH:c * PSH + SH] = x[order[c]].T
    per_layer = {}
    kpos_list = []
    prev_perm = None
    for li, l in enumerate([1, 2, 3]):
        din, dout = DIMS[li]
        Wl = np.asarray(inputs[f"Wl{l}"], np.float32)
        Wr = np.asarray(inputs[f"Wr{l}"], np.float32)
        bl = np.asarray(inputs[f"bl{l}"], np.float32)
        br = np.asarray(inputs[f"br{l}"], np.float32)
        att = np.asarray(inputs[f"att{l}"], np.float32)
        b_l = np.asarray(inputs[f"b{l}"], np.float32)
        perm = np.argsort(att < 0, kind='stable')
        kpos = int((att[perm] >= 0).sum())
        s = np.abs(att[perm])
        s_safe = np.where(s == 0, 1.0, s)
        if prev_perm is not None:
            Wl = Wl[prev_perm]
            Wr = Wr[prev_perm]
        Wlp = np.zeros((din, 64), np.float32)
        Wlp[:, :dout] = Wl[:, perm] * s
        Wrp = np.zeros((din, 64), np.float32)
        Wrp[:, :dout] = Wr[:, perm] * s
        bxr = np.zeros((1, 64), np.float32)
        bxr[0, :dout] = (bl + br)[perm] * s
        invs = np.zeros((1, 64), np.float32)
        invs[0, :dout] = 1.0 / s_safe
        ob = (bl + b_l)[perm]
        if li == 2:
            obias = np.zeros((1, 64), np.float32)
            obias[0, :dout] = ob
        else:
            obias = ob.reshape(dout, 1).astype(np.float32)
        magic = np.zeros((1, 64), np.float32)
        magic[0, :dout] = np.where(np.arange(dout) < kpos, -1000.0, 1000.0)
        per_layer[li] = dict(Wl=Wlp, Wr=Wrp, bxr=bxr, invs=invs, obias=obias,
                             magic=magic, perm=perm)
        kpos_list.append(kpos)
        prev_perm = perm
    return xT, per_layer, kpos_list


_CACHE = {}


def kernel(**inputs):
    global LAST_EXEC_NS
    from concourse import bass_utils

    edge_index = np.asarray(inputs["edge_index"])
    key = "prog"
    if key not in _CACHE:
        idx_flat, meta = _preprocess(edge_index)
        xT, per_layer, kpos_list = _prep_inputs(inputs, meta)
        nc = _build_program(meta, kpos_list)
        _CACHE[key] = (nc, idx_flat, meta, xT, per_layer)
    nc, idx_flat, meta, xT, per_layer = _CACHE[key]

    in_maps = []
    for c in range(NCORES):
        blk_base, Dtot = meta['blk_base'], meta['Dtot']
        parts = []
        for b in range(NBLK):
            bb, dt = int(blk_base[b]), int(Dtot[b])
            parts.append(idx_flat[c][:, 8 * bb:8 * (bb + dt)].reshape(-1))
        idx_c = np.concatenate(parts).reshape(1, -1)
        im = {"xT": xT, "xTloc": xT[:, c * PSH:(c + 1) * PSH].copy(),
              "idxf": idx_c}
        for li in range(3):
            pl = per_layer[li]
            im[f"Wl{li}"] = pl["Wl"]
            im[f"Wr{li}"] = pl["Wr"]
            im[f"bxr{li}"] = pl["bxr"]
            im[f"invs{li}"] = pl["invs"]
            im[f"obias{li}"] = pl["obias"]
            im[f"magic{li}"] = pl["magic"]
        in_maps.append(im)

    res = bass_utils.run_bass_kernel_spmd(
        nc, in_maps, core_ids=list(range(NCORES)), trace=TRACE)
    LAST_EXEC_NS = res.exec_time_ns

    perm3 = per_layer[2]["perm"]
    out = np.zeros((N, 64), np.float32)
    for c in range(NCORES):
        rows = res.results[c]["out"][:SH]
        out[meta['order_per_core'][c]] = rows
    final = np.empty((N, 64), np.float32)
    final[:, perm3] = out
    return final



# revision 2
# speedup vs baseline: 3.2930x; 3.2930x over previous
"""GATv2 3-layer kernel for 8 TRN2 NeuronCores (Bass/Tile).

Dst-sharded: each core owns 12500 dst nodes, replicates the tiny dense
transforms for all nodes into a local DRAM gather table, then runs a
dst-major edge phase (dma_gather of per-edge source rows, DVE softmax +
weighted sum), PE-transposes layer outputs and AllGathers them between
layers.

Perf structure: the per-edge table gather is SWDGE (gpsimd Q7 descriptor
generation) bound. The 4 chunk-gathers per block are striped across the
4 SWDGE queues (queue ch runs on Q7 core pair ch) which pipelines
descriptor generation ~3x. Nodes are grouped into 128-row blocks by
max-per-chunk degree so the per-(block,chunk) padded width (max over
8 cores x 128 partitions) stays near the mean. Padded slots point at
per-chunk magic rows (+-1000) so exp -> 0; att is folded into the
weights (u = |att|*(xl+xr)) with a sign-split min/max leaky-relu.
"""
import sys

sys.path.insert(0, "/opt/trn_rl_repo")

import numpy as np

N = 100000
NCORES = 8
SH = 12500
PSH = 12544                 # 98 * 128
NBLK = 98
NN = NCORES * PSH           # 100352
CSTRIDE = 25089             # chunk stride in table rows (incl magic row)
CNODES = 25088              # real rows per chunk (2 core shards)
MAGIC_LOCAL = CNODES
NCHUNK = 4
NTAB = NCHUNK * CSTRIDE
DIMS = [(11, 16), (16, 32), (32, 64)]
GRP = 896                   # dense-phase node group (7*128)
NGRP = PSH // GRP           # 14

TRACE = False
LAST_EXEC_NS = None


def _preprocess(edge_index):
    src = np.concatenate([edge_index[0].astype(np.int64), np.arange(N, dtype=np.int64)])
    dst = np.concatenate([edge_index[1].astype(np.int64), np.arange(N, dtype=np.int64)])
    node_owner = np.arange(N) // SH
    node_chunk = node_owner // 2

    cnt = np.zeros((N, NCHUNK), dtype=np.int32)
    np.add.at(cnt, (dst, node_chunk[src]), 1)

    localpos = np.empty(N, dtype=np.int64)
    order_per_core = []
    for c in range(NCORES):
        nodes = np.arange(c * SH, (c + 1) * SH)
        cc = cnt[nodes]
        o = np.lexsort((cc.sum(1), cc.max(1)))[::-1]
        nodes = nodes[o]
        order_per_core.append(nodes)
        localpos[nodes] = np.arange(SH)

    tabrow = (node_owner // 2) * CSTRIDE + (node_owner % 2) * PSH + localpos

    cntp = np.zeros((NCORES, NBLK, 128, NCHUNK), dtype=np.int32)
    for c in range(NCORES):
        cc = cnt[order_per_core[c]]
        cc = np.concatenate([cc, np.zeros((PSH - SH, NCHUNK), np.int32)], 0)
        cntp[c] = cc.reshape(NBLK, 128, NCHUNK)
    D = cntp.max(axis=(0, 2)).astype(np.int64)      # [NBLK, NCHUNK]
    Dtot = D.sum(axis=1)                            # [NBLK]
    SDT = int(Dtot.sum())

    blk_base = np.r_[0, np.cumsum(Dtot)][:-1]
    coloff = np.zeros((NBLK, NCHUNK), dtype=np.int64)
    for b in range(NBLK):
        coloff[b] = blk_base[b] + np.r_[0, np.cumsum(D[b])][:-1]

    # slot grid [core, 128, SDT], value = chunk-local table row of src
    ecore = dst // SH
    edl = localpos[dst]
    eblk, epart = edl // 128, edl % 128
    echunk = node_chunk[src]
    eval_loc = tabrow[src] - echunk * CSTRIDE
    key = ((ecore * NBLK + eblk) * 128 + epart) * NCHUNK + echunk
    eo = np.argsort(key, kind='stable')
    keys, vals = key[eo], eval_loc[eo]
    grp_start = np.r_[0, np.flatnonzero(np.diff(keys)) + 1]
    grp_len = np.diff(np.r_[grp_start, len(keys)])
    jidx = np.arange(len(keys)) - np.repeat(grp_start, grp_len)
    kc = keys % NCHUNK
    kp = (keys // NCHUNK) % 128
    kb = (keys // (NCHUNK * 128)) % NBLK
    kcore = keys // (NCHUNK * 128 * NBLK)
    slots = np.full((NCORES, 128, SDT), MAGIC_LOCAL, dtype=np.int64)
    slots[kcore, kp, coloff[kb, kc] + jidx] = vals

    # wrapped int16 per (block, chunk) call, concatenated along free dim
    idx_flat = np.empty((NCORES, 128, 8 * SDT), dtype=np.int16)
    for b in range(NBLK):
        for ch in range(NCHUNK):
            w = int(D[b, ch])
            if w == 0:
                continue
            cs = int(coloff[b, ch])
            sub = slots[:, :, cs:cs + w]                          # [NC,128,w]
            lst = sub.transpose(0, 2, 1).reshape(NCORES, w * 128)  # pos=j*128+p
            wr = lst.reshape(NCORES, w * 8, 16).transpose(0, 2, 1)
            wr = np.tile(wr, (1, 8, 1))
            idx_flat[:, :, 8 * cs:8 * (cs + w)] = wr.astype(np.int16)

    meta = dict(D=D, Dtot=Dtot, coloff=coloff, blk_base=blk_base,
                order_per_core=order_per_core, SDT=SDT)
    return idx_flat, meta


def _build_program(meta, kpos_list):
    import concourse.bass as bass
    import concourse.bacc as bacc
    import concourse.tile as tile
    import concourse.mybir as mybir
    from concourse import masks

    D, Dtot, coloff = meta['D'], meta['Dtot'], meta['coloff']
    blk_base, SDT = meta['blk_base'], meta['SDT']
    f32 = mybir.dt.float32
    i16 = mybir.dt.int16
    AF = mybir.ActivationFunctionType
    OP = mybir.AluOpType
    AX = mybir.AxisListType

    nc = bacc.Bacc("TRN2", target_bir_lowering=False, debug=False,
                   num_devices=NCORES, num_swdge_queues=4)
    t_xT = nc.dram_tensor("xT", [11, NN], f32, kind="ExternalInput")
    t_xTloc = nc.dram_tensor("xTloc", [11, PSH], f32, kind="ExternalInput")
    t_idx = nc.dram_tensor("idxf", [1, 128 * 8 * SDT], i16, kind="ExternalInput")
    t_Wl, t_Wr, t_bxr, t_invs, t_obias, t_magic = [], [], [], [], [], []
    for l in range(3):
        din, dout = DIMS[l]
        t_Wl.append(nc.dram_tensor(f"Wl{l}", [din, 64], f32, kind="ExternalInput"))
        t_Wr.append(nc.dram_tensor(f"Wr{l}", [din, 64], f32, kind="ExternalInput"))
        t_bxr.append(nc.dram_tensor(f"bxr{l}", [1, 64], f32, kind="ExternalInput"))
        t_invs.append(nc.dram_tensor(f"invs{l}", [1, 64], f32, kind="ExternalInput"))
        shape = [1, 64] if l == 2 else [dout, 1]
        t_obias.append(nc.dram_tensor(f"obias{l}", shape, f32, kind="ExternalInput"))
        t_magic.append(nc.dram_tensor(f"magic{l}", [1, 64], f32, kind="ExternalInput"))
    t_out = nc.dram_tensor("out", [PSH, 64], f32, kind="ExternalOutput")

    qctr = [0]

    def nextq():
        q = qctr[0] % 4
        qctr[0] += 1
        return q

    with tile.TileContext(nc) as tc:
        with (tc.tile_pool(name="const", bufs=1) as cpool,
              tc.tile_pool(name="resident", bufs=1) as rpool,
              tc.tile_pool(name="dram", bufs=1, space="DRAM") as dpool,
              tc.tile_pool(name="work", bufs=2) as wpool,
              tc.tile_pool(name="upool", bufs=3) as upool,
              tc.tile_pool(name="feed", bufs=3) as fpool,
              tc.tile_pool(name="small", bufs=4) as spool,
              tc.tile_pool(name="stage", bufs=3) as stpool,
              tc.tile_pool(name="psum", bufs=2, space="PSUM") as ppool,
              tc.tile_pool(name="psumT", bufs=2, space="PSUM") as ppoolT):

            t_tab = [dpool.tile([NTAB, 64], f32, name=f"tab{l}") for l in range(3)]
            t_agin = [dpool.tile([DIMS[l][1], PSH], f32,
                                 name=f"agin{l}") for l in range(2)]
            t_agout = [dpool.tile([NCORES, DIMS[l][1], PSH], f32,
                                  addr_space="Shared", name=f"agout{l}")
                       for l in range(2)]

            ident = cpool.tile([128, 128], f32)
            masks.make_identity(nc, ident[:, :])
            ones_row = cpool.tile([1, 128], f32)
            nc.vector.memset(ones_row[:, :], 1.0)

            def replicate_row(src_row, name):
                ps = ppoolT.tile([128, 64], f32, tag="repl")
                nc.tensor.matmul(ps[:, :], ones_row[:, :], src_row[:, :])
                rep = cpool.tile([128, 64], f32, name=name)
                nc.scalar.activation(rep[:, :], ps[:, :], AF.Copy)
                return rep

            c_bxr, c_invs, c_obias, c_W = [], [], [], []
            for l in range(3):
                din = DIMS[l][0]
                r = cpool.tile([1, 64], f32, name=f"r1_{l}")
                nc.sync.dma_start(r[:, :], t_bxr[l][:, :])
                c_bxr.append(replicate_row(r, f"bxr_{l}"))
                r2 = cpool.tile([1, 64], f32, name=f"r2_{l}")
                nc.sync.dma_start(r2[:, :], t_invs[l][:, :])
                c_invs.append(replicate_row(r2, f"invs_{l}"))
                if l == 2:
                    r3 = cpool.tile([1, 64], f32, name=f"r3_{l}")
                    nc.sync.dma_start(r3[:, :], t_obias[l][:, :])
                    c_obias.append(replicate_row(r3, f"obias_{l}"))
                else:
                    col = cpool.tile([DIMS[l][1], 1], f32, name=f"obias_{l}")
                    nc.sync.dma_start(col[:, :], t_obias[l][:, :])
                    c_obias.append(col)
                mg = cpool.tile([1, 64], f32, name=f"mg_{l}")
                nc.sync.dma_start(mg[:, :], t_magic[l][:, :])
                for ch in range(NCHUNK):
                    row = ch * CSTRIDE + CNODES
                    nc.sync.dma_start(t_tab[l][row:row + 1, :], mg[:, :])
                wl = cpool.tile([din, 64], f32, name=f"cWl{l}")
                nc.sync.dma_start(wl[:, :], t_Wl[l][:, :])
                wr = cpool.tile([din, 64], f32, name=f"cWr{l}")
                nc.sync.dma_start(wr[:, :], t_Wr[l][:, :])
                c_W.append((wl, wr))

            xr_res = rpool.tile([128, NBLK * 64], f32)

            for l in range(3):
                din, dout = DIMS[l]
                kpos = kpos_list[l]
                wl_t, wr_t = c_W[l]

                # ---- dense: xl'' table for all nodes ----
                for G in range(NCORES * NGRP):
                    shard, g = divmod(G, NGRP)
                    hsl = fpool.tile([din, GRP], f32, tag="hsl")
                    if l == 0:
                        nc.sync.dma_start(
                            hsl[:, :],
                            t_xT[:, shard * PSH + g * GRP:shard * PSH + (g + 1) * GRP])
                    else:
                        nc.sync.dma_start(
                            hsl[:, :],
                            t_agout[l - 1][shard, :, g * GRP:(g + 1) * GRP])
                    ps = ppool.tile([128, 448], f32, tag="psd")
                    for j in range(7):
                        nc.tensor.matmul(ps[:, j * 64:(j + 1) * 64],
                                         hsl[:, j * 128:(j + 1) * 128], wl_t[:, :])
                    sb = wpool.tile([128, 448], f32, tag="sbd")
                    nc.scalar.activation(sb[:, :], ps[:, :], AF.Copy)
                    row0 = (shard // 2) * CSTRIDE + (shard % 2) * PSH + g * GRP
                    nc.sync.dma_start(
                        t_tab[l][row0:row0 + GRP, :].rearrange(
                            "(j p) k -> p j k", p=128),
                        sb.rearrange("p (j k) -> p j k", k=64))

                # ---- dense: xr'' for local shard into xr_res ----
                for g in range(NGRP):
                    hsl = fpool.tile([din, GRP], f32, tag="hsl")
                    if l == 0:
                        nc.sync.dma_start(hsl[:, :],
                                          t_xTloc[:, g * GRP:(g + 1) * GRP])
                    else:
                        nc.sync.dma_start(hsl[:, :],
                                          t_agin[l - 1][0:din, g * GRP:(g + 1) * GRP])
                    ps = ppool.tile([128, 448], f32, tag="psd")
                    for j in range(7):
                        nc.tensor.matmul(ps[:, j * 64:(j + 1) * 64],
                                         hsl[:, j * 128:(j + 1) * 128], wr_t[:, :])
                    nc.vector.tensor_tensor(
                        xr_res.rearrange("p (b k) -> p b k", k=64)[:, 7 * g:7 * g + 7, :],
                        ps.rearrange("p (b k) -> p b k", k=64),
                        c_bxr[l].unsqueeze(1).broadcast_to((128, 7, 64)),
                        OP.add)

                # ---- edge phase ----
                for b in range(NBLK):
                    dt = int(Dtot[b])
                    bb = int(blk_base[b])
                    idx_t = fpool.tile([128, 8 * dt], i16, tag="idx")
                    nc.sync.dma_start(
                        idx_t[:, :],
                        t_idx[0, 128 * 8 * bb:128 * 8 * (bb + dt)].rearrange(
                            "(p f) -> p f", p=128))
                    xr_blk = xr_res[:, b * 64:(b + 1) * 64]
                    us = []
                    for ch in range(NCHUNK):
                        w = int(D[b, ch])
                        if w == 0:
                            us.append(None)
                            continue
                        off = int(coloff[b, ch] - blk_base[b])
                        u = upool.tile([128, w * 64], f32, tag=f"u{ch}")
                        nc.gpsimd.dma_gather(
                            u.rearrange("p (d k) -> p d k", d=w),
                            t_tab[l][ch * CSTRIDE:(ch + 1) * CSTRIDE, :],
                            idx_t[:, 8 * off:8 * (off + w)],
                            num_idxs=128 * w, num_idxs_reg=128 * w,
                            elem_size=64, single_packet=False,
                            queue_num=nextq())
                        us.append(u)
                    e = spool.tile([128, dt], f32, tag="e")
                    eN = spool.tile([128, dt], f32, tag="eN")
                    lrs = []
                    for ch in range(NCHUNK):
                        w = int(D[b, ch])
                        if w == 0:
                            lrs.append(None)
                            continue
                        off = int(coloff[b, ch] - blk_base[b])
                        ur = us[ch].rearrange("p (d k) -> p d k", d=w)[:, :, 0:dout]
                        lr = upool.tile([128, w * dout], f32, tag=f"lr{ch}")
                        lrr = lr.rearrange("p (d k) -> p d k", d=w)
                        lrs.append(lrr)
                        nc.vector.tensor_tensor(
                            ur, ur,
                            xr_blk[:, 0:dout].unsqueeze(1).broadcast_to(
                                (128, w, dout)),
                            OP.add)
                        if kpos > 0:
                            nc.vector.scalar_tensor_tensor(
                                lrr[:, :, 0:kpos], ur[:, :, 0:kpos], 0.2,
                                ur[:, :, 0:kpos], OP.mult, OP.max)
                            nc.vector.tensor_reduce(
                                e[:, off:off + w], lrr[:, :, 0:kpos],
                                AX.X, OP.add)
                        if kpos < dout:
                            nc.vector.scalar_tensor_tensor(
                                lrr[:, :, kpos:dout], ur[:, :, kpos:dout], 0.2,
                                ur[:, :, kpos:dout], OP.mult, OP.max)
                            nc.vector.tensor_reduce(
                                eN[:, off:off + w], lrr[:, :, kpos:dout],
                                AX.X, OP.add)
                    if 0 < kpos < dout:
                        nc.vector.tensor_tensor(e[:, :], e[:, :], eN[:, :],
                                                OP.subtract)
                    elif kpos == 0:
                        nc.vector.tensor_scalar_mul(e[:, :], eN[:, :], -1.0)
                    m = spool.tile([128, 1], f32, tag="m")
                    nc.vector.tensor_reduce(m[:, :], e[:, :], AX.X, OP.max)
                    negm = spool.tile([128, 1], f32, tag="negm")
                    nc.vector.tensor_scalar_mul(negm[:, :], m[:, :], -1.0)
                    p = spool.tile([128, dt], f32, tag="p")
                    nc.scalar.activation(p[:, :], e[:, :], AF.Exp,
                                         bias=negm[:, :])
                    den = spool.tile([128, 1], f32, tag="den")
                    nc.vector.tensor_reduce(den[:, :], p[:, :], AX.X, OP.add)
                    rden = spool.tile([128, 1], f32, tag="rden")
                    nc.vector.reciprocal(rden[:, :], den[:, :])
                    outU = spool.tile([128, dout], f32, tag="outU")
                    first = True
                    for ch in range(NCHUNK):
                        w = int(D[b, ch])
                        if w == 0:
                            continue
                        off = int(coloff[b, ch] - blk_base[b])
                        ur = us[ch].rearrange("p (d k) -> p d k", d=w)[:, :, 0:dout]
                        wg3 = lrs[ch]
                        nc.vector.tensor_tensor(
                            wg3, ur,
                            p[:, off:off + w].unsqueeze(2).broadcast_to(
                                (128, w, dout)), OP.mult)
                        tgt = outU if first else spool.tile([128, dout], f32,
                                                            tag="outC")
                        nc.vector.tensor_reduce(
                            tgt[:, :],
                            wg3.rearrange("p d k -> p k d"),
                            AX.X, OP.add)
                        if not first:
                            nc.vector.tensor_tensor(outU[:, :], outU[:, :],
                                                    tgt[:, :], OP.add)
                        first = False
                    o1 = spool.tile([128, dout], f32, tag="o1")
                    nc.vector.scalar_tensor_tensor(
                        o1[:, :], outU[:, :], rden[:, :], xr_blk[:, 0:dout],
                        OP.mult, OP.subtract)
                    o2 = spool.tile([128, dout], f32, tag="o2")
                    nc.vector.tensor_tensor(o2[:, :], o1[:, :],
                                            c_invs[l][:, 0:dout], OP.mult)
                    if l < 2:
                        trp = ppoolT.tile([64, 128], f32, tag="trp")
                        nc.tensor.transpose(trp[0:dout, :], o2[:, :], ident[:, :])
                        hst = stpool.tile([64, 128], f32, tag="hst")
                        nc.scalar.activation(
                            hst[0:dout, :],
                            trp[0:dout, :], AF.Relu, bias=c_obias[l][:, :])
                        nc.sync.dma_start(
                            t_agin[l][:, b * 128:(b + 1) * 128],
                            hst[0:dout, :])
                    else:
                        o3 = spool.tile([128, 64], f32, tag="o3")
                        nc.vector.tensor_tensor(o3[:, :], o2[:, :],
                                                c_obias[l][:, :], OP.add)
                        nc.sync.dma_start(t_out[b * 128:(b + 1) * 128, :],
                                          o3[:, :])

                if l < 2:
                    from concourse.bass import mybir as _mb
                    nc.gpsimd.collective_compute(
                        "AllGather", OP.bypass,
                        replica_groups=[list(range(NCORES))],
                        ins=[t_agin[l].opt()], outs=[t_agout[l].opt()])
    nc.compile()
    return nc


def _prep_inputs(inputs, meta):
    x = np.asarray(inputs["x"], np.float32)
    order = meta['order_per_core']
    xT = np.zeros((11, NN), np.float32)
    for c in range(NCORES):
        xT[:, c * PSH:c * PSH + SH] = x[order[c]].T
    per_layer = {}
    kpos_list = []
    prev_perm = None
    for li, l in enumerate([1, 2, 3]):
        din, dout = DIMS[li]
        Wl = np.asarray(inputs[f"Wl{l}"], np.float32)
        Wr = np.asarray(inputs[f"Wr{l}"], np.float32)
        bl = np.asarray(inputs[f"bl{l}"], np.float32)
        br = np.asarray(inputs[f"br{l}"], np.float32)
        att = np.asarray(inputs[f"att{l}"], np.float32)
        b_l = np.asarray(inputs[f"b{l}"], np.float32)
        perm = np.argsort(att < 0, kind='stable')
        kpos = int((att[perm] >= 0).sum())
        s = np.abs(att[perm])
        s_safe = np.where(s == 0, 1.0, s)
        if prev_perm is not None:
            Wl = Wl[prev_perm]
            Wr = Wr[prev_perm]
        Wlp = np.zeros((din, 64), np.float32)
        Wlp[:, :dout] = Wl[:, perm] * s
        Wrp = np.zeros((din, 64), np.float32)
        Wrp[:, :dout] = Wr[:, perm] * s
        bxr = np.zeros((1, 64), np.float32)
        bxr[0, :dout] = (bl + br)[perm] * s
        invs = np.zeros((1, 64), np.float32)
        invs[0, :dout] = 1.0 / s_safe
        ob = (bl + b_l)[perm]
        if li == 2:
            obias = np.zeros((1, 64), np.float32)
            obias[0, :dout] = ob
        else:
            obias = ob.reshape(dout, 1).astype(np.float32)
        magic = np.zeros((1, 64), np.float32)
        magic[0, :dout] = np.where(np.arange(dout) < kpos, -1000.0, 1000.0)
        per_layer[li] = dict(Wl=Wlp, Wr=Wrp, bxr=bxr, invs=invs, obias=obias,
                             magic=magic, perm=perm)
        kpos_list.append(kpos)
        prev_perm = perm
    return xT, per_layer, kpos_list


_CACHE = {}


def kernel(**inputs):
    global LAST_EXEC_NS
    from concourse import bass_utils

    edge_index = np.asarray(inputs["edge_index"])
    key = "prog"
    if key not in _CACHE:
        idx_flat, meta = _preprocess(edge_index)
        xT, per_layer, kpos_list = _prep_inputs(inputs, meta)
        nc = _build_program(meta, kpos_list)
        _CACHE[key] = (nc, idx_flat, meta, xT, per_layer)
    nc, idx_flat, meta, xT, per_layer = _CACHE[key]

    in_maps = []
    for c in range(NCORES):
        blk_base, Dtot = meta['blk_base'], meta['Dtot']
        parts = []
        for b in range(NBLK):
            bb, dt = int(blk_base[b]), int(Dtot[b])
            parts.append(idx_flat[c][:, 8 * bb:8 * (bb + dt)].reshape(-1))
        idx_c = np.concatenate(parts).reshape(1, -1)
        im = {"xT": xT, "xTloc": xT[:, c * PSH:(c + 1) * PSH].copy(),
              "idxf": idx_c}
        for li in range(3):
            pl = per_layer[li]
            im[f"Wl{li}"] = pl["Wl"]
            im[f"Wr{li}"] = pl["Wr"]
            im[f"bxr{li}"] = pl["bxr"]
            im[f"invs{li}"] = pl["invs"]
            im[f"obias{li}"] = pl["obias"]
            im[f"magic{li}"] = pl["magic"]
        in_maps.append(im)

    res = bass_utils.run_bass_kernel_spmd(
        nc, in_maps, core_ids=list(range(NCORES)), trace=TRACE)
    LAST_EXEC_NS = res.exec_time_ns

    perm3 = per_layer[2]["perm"]
    out = np.zeros((N, 64), np.float32)
    for c in range(NCORES):
        rows = res.results[c]["out"][:SH]
        out[meta['order_per_core'][c]] = rows
    final = np.empty((N, 64), np.float32)
    final[:, perm3] = out
    return final
